# revision 21
# baseline (speedup 1.0000x reference)
"""DynaConv Trainium2 kernel.

Network (per pixel p, reference semantics):
    feat = unfold3x3(x)                       # [144] = (c major, ij minor)
    hid  = tanh(W1 @ feat + b1)               # [32]
    kern = (W2 @ hid + b2).reshape(32, 9)     # [32, 9]
    s    = channel-sum of patch               # [9]
    out  = kern @ s + bias                    # [32]

Kernel strategy (per core, channels-in-partitions):
  - 8 cores: core = 2*b + h handles batch b, H-half h (128 rows x 256 cols
    = 32768 pixels).  Host pads x spatially and ships a [16, 130, 260] slab.
  - SBUF slab layout: partition (16*s + c) holds padded rows [16s, 16s+18)
    of channel c ("slot" s in 0..7), so every 2-row x 256-col tile reads its
    3x3 shifted windows as plain strided APs.
  - stage2 (conv3x3 + patch-sum): 9 accumulating matmuls (K=32 with the
    inactive 16 rows zeroed in lhsT, M=41 = 32 z-cols + 9 s-cols, N=512).
  - tanh+b1 on ACT -> hid[32,512]; s rows copied to SBUF.
  - U build: hid_rep / s_rep via replication matmuls, elementwise products
    on DVE -> U[297,512] (rows k*32+g = hid_g*s_k, plus 9 raw s rows).
  - stage5: out[32,512] = L2^T @ U (3 accumulating matmuls K=128/128/41),
    which applies W2 and the b2*s term in one contraction.
  - +bias via ACT Identity, DMA out.
"""

import os
import numpy as np

B, C, H, W = 4, 16, 256, 256
O = 32
NCORES = 8
HWPAD = 260  # padded width: col 0 = left pad, 1..256 data, 257+ pad
SLAB_ROWS = 130
N = 512  # pixels per tile = 2 rows x 256
FP32R = os.environ.get("DYNA_FP32R", "1") == "1"
BENCH_REPS = int(os.environ.get("DYNA_BENCH_REPS", "1"))

_cache = {}


def _build_weights(W1, b1, W2, b2, bias):
    """Host-side packing of all weight operands (pure layout, no math)."""
    f4 = np.float32
    # stage2 lhsT: per (parity p, shift ij): [32, 41]; replicated per 32-row
    # group q so lhsT.base == rhs.base for the default tile_position path.
    W1X = np.zeros((128, 3, 41), f4)
    cs = np.arange(16)
    for G in range(2):
        for j in range(3):
            rows = 64 * G + 16 * j + cs
            for i in range(3):
                W1X[rows, i, :32] = W1[:, cs * 9 + 3 * i + j].T  # [16c, 32o]
                W1X[rows, i, 32 + 3 * i + j] = 1.0  # patch channel-sum fold
    W1X = W1X.reshape(128, 123)

    # stage5 lhsT chunks: U rows (k*32+g) k=0..3 | k=4..7 | (k=8, s rows)
    W2r = W2.reshape(O, 9, O)  # [o, k, g]
    L2 = np.zeros((128, 3, 32), f4)
    for k in range(4):
        L2[32 * k:32 * k + 32, 0, :] = W2r[:, k, :].T
        L2[32 * k:32 * k + 32, 1, :] = W2r[:, 4 + k, :].T
    L2[0:32, 2, :] = W2r[:, 8, :].T
    L2[32:41, 2, :] = b2.reshape(32, 9).T  # [kk, o]
    L2 = L2.reshape(128, 96)

    # replication matmul lhsTs
    REP = np.zeros((128, 416), f4)
    g = np.arange(32)
    for kp in range(4):
        REP[g, 32 * kp + g] = 1.0                      # hid_rep  (rows 0..31)
        REP[32 + kp, 128 + 32 * kp: 160 + 32 * kp] = 1.0   # s_rep a  (rows 32..40)
        REP[36 + kp, 256 + 32 * kp: 288 + 32 * kp] = 1.0   # s_rep b
    REP[40, 384:416] = 1.0                             # s_rep c (k=8)

    BIA = np.zeros((32, 2), f4)
    BIA[:, 0] = b1
    BIA[:, 1] = bias
    return {"W1X": W1X, "L2": L2, "REP": REP, "BIA": BIA}


def _build_nc():
    from contextlib import ExitStack

    import concourse.bass as bass
    import concourse.mybir as mybir
    import concourse.tile as tile
    from concourse import bacc

    f32 = mybir.dt.float32
    # matmul-operand dtype: float32r streams 1 row/cycle (vs 4 for float32);
    # producers (DMA/ACT/DVE) must then declare float32r outputs so the
    # rounding happens at write time (walrus birverifier enforces this).
    mdt = mybir.dt.float32r if FP32R else f32

    def mmcast(ap):
        return ap

    nc = bacc.Bacc("TRN2", target_bir_lowering=False, debug=False)
    xs = nc.dram_tensor("xs", [16, SLAB_ROWS, HWPAD], mdt, kind="ExternalInput").ap()
    w1x_d = nc.dram_tensor("W1X", [128, 123], mdt, kind="ExternalInput").ap()
    l2_d = nc.dram_tensor("L2", [128, 96], mdt, kind="ExternalInput").ap()
    rep_d = nc.dram_tensor("REP", [128, 416], mdt, kind="ExternalInput").ap()
    bia_d = nc.dram_tensor("BIA", [32, 2], f32, kind="ExternalInput").ap()
    ys = nc.dram_tensor("ys", [32, 128, 256], f32, kind="ExternalOutput").ap()

    with tile.TileContext(nc) as tc, ExitStack() as ctx:
        const = ctx.enter_context(tc.tile_pool(name="const", bufs=1))
        sb = ctx.enter_context(tc.tile_pool(name="sb", bufs=3))
        ps2 = ctx.enter_context(tc.tile_pool(name="ps2", bufs=2, space="PSUM"))
        psrep = ctx.enter_context(tc.tile_pool(name="psrep", bufs=2, space="PSUM"))
        pshr = ctx.enter_context(tc.tile_pool(name="pshr", bufs=1, space="PSUM"))
        psout = ctx.enter_context(tc.tile_pool(name="psout", bufs=2, space="PSUM"))

        slab = const.tile([128, 66 * 256], mdt)
        w1x = const.tile([128, 123], mdt)
        l2 = const.tile([128, 96], mdt)
        rep = const.tile([128, 416], mdt)
        bia = const.tile([32, 2], f32)

        # x3 slab: partition 64G+16j+c holds rows [64G, 64G+66) of channel c
        # with the j-th column shift baked in (width 256).
        for G in range(2):
            for j in range(3):
                in_ap = bass.AP(
                    xs.tensor, 64 * G * HWPAD + j,
                    [[SLAB_ROWS * HWPAD, 16], [HWPAD, 66], [1, 256]])
                nc.sync.dma_start(
                    slab[64 * G + 16 * j:64 * G + 16 * j + 16, :].rearrange(
                        "c (r w) -> c r w", r=66, w=256),
                    in_ap,
                )
        nc.sync.dma_start(w1x[:], w1x_d)
        nc.sync.dma_start(l2[:], l2_d)
        nc.sync.dma_start(rep[:], rep_d)
        nc.sync.dma_start(bia[:], bia_d)

        slab3 = slab[:].rearrange("p (r w) -> p r w", r=66, w=256)

        from contextlib import nullcontext
        loop_ctx = (tc.For_i(0, BENCH_REPS, 1) if BENCH_REPS > 1
                    else nullcontext())
        with loop_ctx:
            _tile_body(nc, tc, mybir, slab3, w1x, l2, rep, bia, ys,
                       sb, ps2, psrep, pshr, psout, mmcast, mdt)

    nc.compile()
    return nc


def _tile_body(nc, tc, mybir, slab3, w1x, l2, rep, bia, ys,
               sb, ps2, psrep, pshr, psout, mmcast, mdt):
    f32 = mybir.dt.float32
    for G in range(2):
            for t in range(32):
                # stage2: 3 row-shift matmuls (K=48 j-replicated) -> psum2
                psum2 = ps2.tile([41, N], f32)
                for i in range(3):
                    rhs = slab3[64 * G:64 * G + 48,
                                2 * t + i: 2 * t + i + 2, :]
                    lhsT = w1x[64 * G:64 * G + 48, i * 41:(i + 1) * 41]
                    nc.tensor.matmul(psum2[:], mmcast(lhsT), mmcast(rhs),
                                     start=(i == 0), stop=(i == 2),
                                     tile_position=(64 * G, 0))

                # hid = tanh(z + b1); s rows go straight into uc
                hid = sb.tile([32, N], mdt, tag="hid")
                uc = sb.tile([48, N], mdt, tag="uc")
                nc.scalar.activation(hid[:], psum2[0:32, :],
                                     mybir.ActivationFunctionType.Tanh,
                                     bias=bia[0:32, 0:1], scale=1.0)
                nc.vector.tensor_copy(uc[32:41, :], psum2[32:41, :])

                # replication matmuls
                hrep = pshr.tile([128, N], f32)
                nc.tensor.matmul(hrep[:], mmcast(rep[0:32, 0:128]),
                                 mmcast(hid[:]), start=True, stop=True,
                                 tile_position=(0, 0))
                hrep_sb = sb.tile([128, N], mdt, tag="hrep_sb")
                nc.scalar.copy(hrep_sb[:], hrep[:])

                ua = sb.tile([128, N], mdt, tag="ua")
                ub = sb.tile([128, N], mdt, tag="ub")
                srep = psrep.tile([128, N], f32, tag="srep")
                nc.tensor.matmul(srep[:], mmcast(rep[32:41, 128:256]),
                                 mmcast(uc[32:41, :]), start=True, stop=True,
                                 tile_position=(32, 0))
                nc.vector.tensor_mul(ua[:], hrep_sb[:], srep[:])
                srep2 = psrep.tile([128, N], f32, tag="srep")
                nc.tensor.matmul(srep2[:], mmcast(rep[32:41, 256:384]),
                                 mmcast(uc[32:41, :]), start=True, stop=True,
                                 tile_position=(32, 0))
                nc.vector.tensor_mul(ub[:], hrep_sb[:], srep2[:])
                srep3 = psrep.tile([32, N], f32, tag="srep")
                nc.tensor.matmul(srep3[:], mmcast(rep[32:41, 384:416]),
                                 mmcast(uc[32:41, :]), start=True, stop=True,
                                 tile_position=(32, 0))
                nc.vector.tensor_mul(uc[0:32, :], hid[:], srep3[:])

                # stage5: out = L2^T @ U  (+ b2*s folded in chunk c)
                outp = psout.tile([32, N], f32)
                nc.tensor.matmul(outp[:], mmcast(l2[0:128, 0:32]),
                                 mmcast(ua[:]), start=True, stop=False,
                                 tile_position=(0, 0))
                nc.tensor.matmul(outp[:], mmcast(l2[0:128, 32:64]),
                                 mmcast(ub[:]), start=False, stop=False,
                                 tile_position=(0, 0))
                nc.tensor.matmul(outp[:], mmcast(l2[0:41, 64:96]),
                                 mmcast(uc[0:41, :]), start=False, stop=True,
                                 tile_position=(0, 0))

                if t % 4 == 0:
                    out_sb4 = sb.tile([32, 4 * N], f32, tag="out_sb4")
                nc.scalar.activation(out_sb4[:, (t % 4) * N:(t % 4 + 1) * N],
                                     outp[:],
                                     mybir.ActivationFunctionType.Identity,
                                     bias=bia[0:32, 1:2], scale=1.0)
                if t % 4 == 3:
                    r0 = 64 * G + 2 * (t - 3)
                    nc.sync.dma_start(
                        ys[:, r0:r0 + 8, :],
                        out_sb4[:].rearrange("o (r w) -> o r w", r=8, w=256),
                    )


def _get_runner():
    """Build (once) a persistent jitted 8-core SPMD callable."""
    if "runner" in _cache:
        return _cache["runner"]

    import jax
    import jax.numpy as jnp
    from jax.sharding import Mesh, PartitionSpec
    from jax.experimental.shard_map import shard_map

    import concourse.mybir as mybir
    from concourse import bass2jax
    from concourse.bass2jax import _bass_exec_p, install_neuronx_cc_hook

    nc = _build_nc()
    install_neuronx_cc_hook()

    partition_name = (nc.partition_id_tensor.name
                      if nc.partition_id_tensor else None)
    in_names, out_names, out_avals, zero_outs = [], [], [], []
    for alloc in nc.m.functions[0].allocations:
        if not isinstance(alloc, mybir.MemoryLocationSet):
            continue
        name = alloc.memorylocations[0].name
        if alloc.kind == "ExternalInput":
            if name != partition_name:
                in_names.append(name)
        elif alloc.kind == "ExternalOutput":
            shape = tuple(alloc.tensor_shape)
            dtype = mybir.dt.np(alloc.dtype)
            out_names.append(name)
            out_avals.append(jax.core.ShapedArray(shape, dtype))
            zero_outs.append(np.zeros(shape, dtype))
    n_params = len(in_names)
    n_outs = len(out_avals)
    all_in_names = in_names + out_names
    if partition_name is not None:
        all_in_names = all_in_names + [partition_name]

    def _body(*args):
        operands = list(args)
        if partition_name is not None:
            operands.append(bass2jax.partition_id_tensor())
        outs = _bass_exec_p.bind(
            *operands,
            out_avals=tuple(out_avals),
            in_names=tuple(all_in_names),
            out_names=tuple(out_names),
            lowering_input_output_aliases=(),
            sim_require_finite=True,
            sim_require_nnan=True,
            nc=nc,
        )
        return tuple(outs)

    devices = jax.devices()[:NCORES]
    mesh = Mesh(np.asarray(devices), ("core",))
    in_specs = (PartitionSpec("core"),) * (n_params + n_outs)
    out_specs = (PartitionSpec("core"),) * n_outs
    donate = tuple(range(n_params, n_params + n_outs))
    sharded = jax.jit(
        shard_map(_body, mesh=mesh, in_specs=in_specs, out_specs=out_specs,
                  check_rep=False),
        donate_argnums=donate, keep_unused=True,
    )

    state = {
        "sharded": sharded, "in_names": in_names, "out_names": out_names,
        "out_avals": out_avals, "zero_outs": zero_outs,
    }

    def run(in_maps):
        concat_in = [
            np.concatenate([np.asarray(in_maps[c][name]) for c in range(NCORES)],
                           axis=0)
            for name in state["in_names"]
        ]
        concat_zeros = [
            np.zeros((NCORES * z.shape[0], *z.shape[1:]), z.dtype)
            for z in state["zero_outs"]
        ]
        out_arrs = state["sharded"](*concat_in, *concat_zeros)
        out_arrs = [np.asarray(a) for a in jax.block_until_ready(out_arrs)]
        return [
            {name: out_arrs[i].reshape(NCORES, *state["out_avals"][i].shape)[c]
             for i, name in enumerate(state["out_names"])}
            for c in range(NCORES)
        ]

    def bench(in_maps, iters=16, reps=4):
        """Async-pipelined dispatches on device-resident inputs; returns
        estimated per-execution wall time in ns (min over reps)."""
        import time as _time

        concat_in = [
            np.concatenate([np.asarray(in_maps[c][name]) for c in range(NCORES)],
                           axis=0)
            for name in state["in_names"]
        ]
        concat_zeros = [
            np.zeros((NCORES * z.shape[0], *z.shape[1:]), z.dtype)
            for z in state["zero_outs"]
        ]
        from jax.sharding import NamedSharding
        sh = NamedSharding(mesh, PartitionSpec("core"))
        dev_in = [jax.device_put(a, sh) for a in concat_in]
        best = None
        for rep in range(reps):
            zsets = [[jax.device_put(z, sh) for z in concat_zeros]
                     for _ in range(iters)]
            jax.block_until_ready(zsets)
            outs = state["sharded"](*dev_in, *zsets[0])  # warm dispatch path
            jax.block_until_ready(outs)
            t0 = _time.perf_counter()
            res = [state["sharded"](*dev_in, *zs) for zs in zsets[1:]]
            jax.block_until_ready(res)
            t1 = _time.perf_counter()
            per = (t1 - t0) / (iters - 1)
            best = per if best is None else min(best, per)
        return best * 1e9

    _cache["runner"] = run
    _cache["bench"] = bench
    run.bench = bench
    return run


def _make_in_maps(x, W1, b1, W2, b2, bias):
    wts = _build_weights(np.asarray(W1, np.float32), np.asarray(b1, np.float32),
                         np.asarray(W2, np.float32), np.asarray(b2, np.float32),
                         np.asarray(bias, np.float32))
    x = np.asarray(x, np.float32)
    xp = np.pad(x, ((0, 0), (0, 0), (1, 1), (1, 3)))  # [4, 16, 258, 260]
    in_maps = []
    for core in range(NCORES):
        b, h = divmod(core, 2)
        slab = np.ascontiguousarray(xp[b, :, 128 * h:128 * h + SLAB_ROWS, :])
        in_maps.append({"xs": slab, **wts})
    return in_maps


def kernel(x, W1, b1, W2, b2, bias):
    run = _get_runner()
    in_maps = _make_in_maps(x, W1, b1, W2, b2, bias)
    results = run(in_maps)
    out = np.empty((B, O, H, W), np.float32)
    for core in range(NCORES):
        b, h = divmod(core, 2)
        out[b, :, 128 * h:128 * h + 128, :] = results[core]["ys"]
    return out


# revision 22
# speedup vs baseline: 17.5608x; 17.5608x over previous
"""DynaConv Trainium2 kernel.

Network (per pixel p, reference semantics):
    feat = unfold3x3(x)                       # [144] = (c major, ij minor)
    hid  = tanh(W1 @ feat + b1)               # [32]
    kern = (W2 @ hid + b2).reshape(32, 9)     # [32, 9]
    s    = channel-sum of patch               # [9]
    out  = kern @ s + bias                    # [32]

Kernel strategy (per core, channels-in-partitions):
  - 8 cores: core = 2*b + h handles batch b, H-half h (128 rows x 256 cols
    = 32768 pixels).  Host pads x spatially and ships a [16, 130, 260] slab.
  - SBUF slab layout: partition (16*s + c) holds padded rows [16s, 16s+18)
    of channel c ("slot" s in 0..7), so every 2-row x 256-col tile reads its
    3x3 shifted windows as plain strided APs.
  - stage2 (conv3x3 + patch-sum): 9 accumulating matmuls (K=32 with the
    inactive 16 rows zeroed in lhsT, M=41 = 32 z-cols + 9 s-cols, N=512).
  - tanh+b1 on ACT -> hid[32,512]; s rows copied to SBUF.
  - U build: hid_rep / s_rep via replication matmuls, elementwise products
    on DVE -> U[297,512] (rows k*32+g = hid_g*s_k, plus 9 raw s rows).
  - stage5: out[32,512] = L2^T @ U (3 accumulating matmuls K=128/128/41),
    which applies W2 and the b2*s term in one contraction.
  - +bias via ACT Identity, DMA out.
"""

import os
import numpy as np

B, C, H, W = 4, 16, 256, 256
O = 32
NCORES = 8
HWPAD = 260  # padded width: col 0 = left pad, 1..256 data, 257+ pad
SLAB_ROWS = 130
N = 512  # pixels per tile = 2 rows x 256
FP32R = os.environ.get("DYNA_FP32R", "1") == "1"
BENCH_REPS = int(os.environ.get("DYNA_BENCH_REPS", "1"))

_cache = {}


def _build_weights(W1, b1, W2, b2, bias):
    """Host-side packing of all weight operands (pure layout, no math)."""
    f4 = np.float32
    # stage2 lhsT: per (parity p, shift ij): [32, 41]; replicated per 32-row
    # group q so lhsT.base == rhs.base for the default tile_position path.
    W1X = np.zeros((128, 3, 41), f4)
    cs = np.arange(16)
    for G in range(2):
        for j in range(3):
            rows = 64 * G + 16 * j + cs
            for i in range(3):
                W1X[rows, i, :32] = W1[:, cs * 9 + 3 * i + j].T  # [16c, 32o]
                W1X[rows, i, 32 + 3 * i + j] = 1.0  # patch channel-sum fold
    W1X = W1X.reshape(128, 123)

    # stage5 lhsT chunks: U rows (k*32+g) k=0..3 | k=4..7 | (k=8, s rows)
    W2r = W2.reshape(O, 9, O)  # [o, k, g]
    L2 = np.zeros((128, 3, 32), f4)
    for k in range(4):
        L2[32 * k:32 * k + 32, 0, :] = W2r[:, k, :].T
        L2[32 * k:32 * k + 32, 1, :] = W2r[:, 4 + k, :].T
    L2[0:32, 2, :] = W2r[:, 8, :].T
    L2[32:41, 2, :] = b2.reshape(32, 9).T  # [kk, o]
    L2 = L2.reshape(128, 96)

    # replication matmul lhsTs
    REP = np.zeros((128, 416), f4)
    g = np.arange(32)
    for kp in range(4):
        REP[g, 32 * kp + g] = 1.0                      # hid_rep  (rows 0..31)
        REP[32 + kp, 128 + 32 * kp: 160 + 32 * kp] = 1.0   # s_rep a  (rows 32..40)
        REP[36 + kp, 256 + 32 * kp: 288 + 32 * kp] = 1.0   # s_rep b
    REP[40, 384:416] = 1.0                             # s_rep c (k=8)

    BIA = np.zeros((32, 2), f4)
    BIA[:, 0] = b1
    BIA[:, 1] = bias
    return {"W1X": W1X, "L2": L2, "REP": REP, "BIA": BIA}


def _build_nc():
    from contextlib import ExitStack

    import concourse.bass as bass
    import concourse.mybir as mybir
    import concourse.tile as tile
    from concourse import bacc

    f32 = mybir.dt.float32
    # matmul-operand dtype: float32r streams 1 row/cycle (vs 4 for float32);
    # producers (DMA/ACT/DVE) must then declare float32r outputs so the
    # rounding happens at write time (walrus birverifier enforces this).
    mdt = mybir.dt.float32r if FP32R else f32

    def mmcast(ap):
        return ap

    nc = bacc.Bacc("TRN2", target_bir_lowering=False, debug=False)
    xs = nc.dram_tensor("xs", [16, SLAB_ROWS, HWPAD], mdt, kind="ExternalInput").ap()
    w1x_d = nc.dram_tensor("W1X", [128, 123], mdt, kind="ExternalInput").ap()
    l2_d = nc.dram_tensor("L2", [128, 96], mdt, kind="ExternalInput").ap()
    rep_d = nc.dram_tensor("REP", [128, 416], mdt, kind="ExternalInput").ap()
    bia_d = nc.dram_tensor("BIA", [32, 2], f32, kind="ExternalInput").ap()
    ys = nc.dram_tensor("ys", [32, 128, 256], f32, kind="ExternalOutput").ap()

    with tile.TileContext(nc) as tc, ExitStack() as ctx:
        const = ctx.enter_context(tc.tile_pool(name="const", bufs=1))
        sb = ctx.enter_context(tc.tile_pool(name="sb", bufs=2))
        ps2 = ctx.enter_context(tc.tile_pool(name="ps2", bufs=2, space="PSUM"))
        psrep = ctx.enter_context(tc.tile_pool(name="psrep", bufs=2, space="PSUM"))
        pshr = ctx.enter_context(tc.tile_pool(name="pshr", bufs=1, space="PSUM"))
        psout = ctx.enter_context(tc.tile_pool(name="psout", bufs=2, space="PSUM"))

        slab = const.tile([128, 66 * 256], mdt)
        w1x = const.tile([128, 123], mdt)
        l2 = const.tile([128, 96], mdt)
        rep = const.tile([128, 416], mdt)
        bia = const.tile([32, 2], f32)

        # x3 slab: partition 64G+16j+c holds rows [64G, 64G+66) of channel c
        # with the j-th column shift baked in (width 256).
        for G in range(2):
            for j in range(3):
                in_ap = bass.AP(
                    xs.tensor, 64 * G * HWPAD + j,
                    [[SLAB_ROWS * HWPAD, 16], [HWPAD, 66], [1, 256]])
                nc.sync.dma_start(
                    slab[64 * G + 16 * j:64 * G + 16 * j + 16, :].rearrange(
                        "c (r w) -> c r w", r=66, w=256),
                    in_ap,
                )
        nc.sync.dma_start(w1x[:], w1x_d)
        nc.sync.dma_start(l2[:], l2_d)
        nc.sync.dma_start(rep[:], rep_d)
        nc.sync.dma_start(bia[:], bia_d)

        slab3 = slab[:].rearrange("p (r w) -> p r w", r=66, w=256)

        from contextlib import nullcontext
        loop_ctx = (tc.For_i(0, BENCH_REPS, 1) if BENCH_REPS > 1
                    else nullcontext())
        with loop_ctx:
            _tile_body(nc, tc, mybir, slab3, w1x, l2, rep, bia, ys,
                       sb, ps2, psrep, pshr, psout, mmcast, mdt)

    nc.compile()
    return nc


def _tile_body(nc, tc, mybir, slab3, w1x, l2, rep, bia, ys,
               sb, ps2, psrep, pshr, psout, mmcast, mdt):
    f32 = mybir.dt.float32
    for G in range(2):
            for t in range(32):
                # stage2: 3 row-shift matmuls (K=48 j-replicated) -> psum2
                psum2 = ps2.tile([41, N], f32)
                for i in range(3):
                    rhs = slab3[64 * G:64 * G + 48,
                                2 * t + i: 2 * t + i + 2, :]
                    lhsT = w1x[64 * G:64 * G + 48, i * 41:(i + 1) * 41]
                    nc.tensor.matmul(psum2[:], mmcast(lhsT), mmcast(rhs),
                                     start=(i == 0), stop=(i == 2),
                                     tile_position=(64 * G, 0))

                # hid = tanh(z + b1); s rows go straight into uc
                hid = sb.tile([32, N], mdt, tag="hid")
                uc = sb.tile([48, N], mdt, tag="uc")
                nc.scalar.activation(hid[:], psum2[0:32, :],
                                     mybir.ActivationFunctionType.Tanh,
                                     bias=bia[0:32, 0:1], scale=1.0)
                nc.vector.tensor_copy(uc[32:41, :], psum2[32:41, :])

                # replication matmuls
                hrep = pshr.tile([128, N], f32)
                nc.tensor.matmul(hrep[:], mmcast(rep[0:32, 0:128]),
                                 mmcast(hid[:]), start=True, stop=True,
                                 tile_position=(0, 0))
                hrep_sb = sb.tile([128, N], mdt, tag="hrep_sb")
                nc.scalar.copy(hrep_sb[:], hrep[:])

                ua = sb.tile([128, N], mdt, tag="ua")
                ub = sb.tile([128, N], mdt, tag="ub")
                srep = psrep.tile([128, N], f32, tag="srep")
                nc.tensor.matmul(srep[:], mmcast(rep[32:41, 128:256]),
                                 mmcast(uc[32:41, :]), start=True, stop=True,
                                 tile_position=(32, 0))
                nc.vector.tensor_mul(ua[:], hrep_sb[:], srep[:])
                srep2 = psrep.tile([128, N], f32, tag="srep")
                nc.tensor.matmul(srep2[:], mmcast(rep[32:41, 256:384]),
                                 mmcast(uc[32:41, :]), start=True, stop=True,
                                 tile_position=(32, 0))
                nc.vector.tensor_mul(ub[:], hrep_sb[:], srep2[:])
                srep3 = psrep.tile([32, N], f32, tag="srep")
                nc.tensor.matmul(srep3[:], mmcast(rep[32:41, 384:416]),
                                 mmcast(uc[32:41, :]), start=True, stop=True,
                                 tile_position=(32, 0))
                nc.vector.tensor_mul(uc[0:32, :], hid[:], srep3[:])

                # stage5: out = L2^T @ U  (+ b2*s folded in chunk c)
                outp = psout.tile([32, N], f32)
                nc.tensor.matmul(outp[:], mmcast(l2[0:128, 0:32]),
                                 mmcast(ua[:]), start=True, stop=False,
                                 tile_position=(0, 0))
                nc.tensor.matmul(outp[:], mmcast(l2[0:128, 32:64]),
                                 mmcast(ub[:]), start=False, stop=False,
                                 tile_position=(0, 0))
                nc.tensor.matmul(outp[:], mmcast(l2[0:41, 64:96]),
                                 mmcast(uc[0:41, :]), start=False, stop=True,
                                 tile_position=(0, 0))

                if t % 4 == 0:
                    out_sb4 = sb.tile([32, 4 * N], f32, tag="out_sb4")
                nc.scalar.activation(out_sb4[:, (t % 4) * N:(t % 4 + 1) * N],
                                     outp[:],
                                     mybir.ActivationFunctionType.Identity,
                                     bias=bia[0:32, 1:2], scale=1.0)
                if t % 4 == 3:
                    r0 = 64 * G + 2 * (t - 3)
                    nc.sync.dma_start(
                        ys[:, r0:r0 + 8, :],
                        out_sb4[:].rearrange("o (r w) -> o r w", r=8, w=256),
                    )


def _get_runner():
    """Build (once) a persistent jitted 8-core SPMD callable."""
    if "runner" in _cache:
        return _cache["runner"]

    import jax
    import jax.numpy as jnp
    from jax.sharding import Mesh, PartitionSpec
    from jax.experimental.shard_map import shard_map

    import concourse.mybir as mybir
    from concourse import bass2jax
    from concourse.bass2jax import _bass_exec_p, install_neuronx_cc_hook

    nc = _build_nc()
    install_neuronx_cc_hook()

    partition_name = (nc.partition_id_tensor.name
                      if nc.partition_id_tensor else None)
    in_names, out_names, out_avals, zero_outs = [], [], [], []
    for alloc in nc.m.functions[0].allocations:
        if not isinstance(alloc, mybir.MemoryLocationSet):
            continue
        name = alloc.memorylocations[0].name
        if alloc.kind == "ExternalInput":
            if name != partition_name:
                in_names.append(name)
        elif alloc.kind == "ExternalOutput":
            shape = tuple(alloc.tensor_shape)
            dtype = mybir.dt.np(alloc.dtype)
            out_names.append(name)
            out_avals.append(jax.core.ShapedArray(shape, dtype))
            zero_outs.append(np.zeros(shape, dtype))
    n_params = len(in_names)
    n_outs = len(out_avals)
    all_in_names = in_names + out_names
    if partition_name is not None:
        all_in_names = all_in_names + [partition_name]

    def _body(*args):
        operands = list(args)
        if partition_name is not None:
            operands.append(bass2jax.partition_id_tensor())
        outs = _bass_exec_p.bind(
            *operands,
            out_avals=tuple(out_avals),
            in_names=tuple(all_in_names),
            out_names=tuple(out_names),
            lowering_input_output_aliases=(),
            sim_require_finite=True,
            sim_require_nnan=True,
            nc=nc,
        )
        return tuple(outs)

    devices = jax.devices()[:NCORES]
    mesh = Mesh(np.asarray(devices), ("core",))
    in_specs = (PartitionSpec("core"),) * (n_params + n_outs)
    out_specs = (PartitionSpec("core"),) * n_outs
    donate = tuple(range(n_params, n_params + n_outs))
    sharded = jax.jit(
        shard_map(_body, mesh=mesh, in_specs=in_specs, out_specs=out_specs,
                  check_rep=False),
        donate_argnums=donate, keep_unused=True,
    )

    state = {
        "sharded": sharded, "in_names": in_names, "out_names": out_names,
        "out_avals": out_avals, "zero_outs": zero_outs,
    }

    def run(in_maps):
        concat_in = [
            np.concatenate([np.asarray(in_maps[c][name]) for c in range(NCORES)],
                           axis=0)
            for name in state["in_names"]
        ]
        concat_zeros = [
            np.zeros((NCORES * z.shape[0], *z.shape[1:]), z.dtype)
            for z in state["zero_outs"]
        ]
        out_arrs = state["sharded"](*concat_in, *concat_zeros)
        out_arrs = [np.asarray(a) for a in jax.block_until_ready(out_arrs)]
        return [
            {name: out_arrs[i].reshape(NCORES, *state["out_avals"][i].shape)[c]
             for i, name in enumerate(state["out_names"])}
            for c in range(NCORES)
        ]

    def bench(in_maps, iters=16, reps=4):
        """Async-pipelined dispatches on device-resident inputs; returns
        estimated per-execution wall time in ns (min over reps)."""
        import time as _time

        concat_in = [
            np.concatenate([np.asarray(in_maps[c][name]) for c in range(NCORES)],
                           axis=0)
            for name in state["in_names"]
        ]
        concat_zeros = [
            np.zeros((NCORES * z.shape[0], *z.shape[1:]), z.dtype)
            for z in state["zero_outs"]
        ]
        from jax.sharding import NamedSharding
        sh = NamedSharding(mesh, PartitionSpec("core"))
        dev_in = [jax.device_put(a, sh) for a in concat_in]
        best = None
        for rep in range(reps):
            zsets = [[jax.device_put(z, sh) for z in concat_zeros]
                     for _ in range(iters)]
            jax.block_until_ready(zsets)
            outs = state["sharded"](*dev_in, *zsets[0])  # warm dispatch path
            jax.block_until_ready(outs)
            t0 = _time.perf_counter()
            res = [state["sharded"](*dev_in, *zs) for zs in zsets[1:]]
            jax.block_until_ready(res)
            t1 = _time.perf_counter()
            per = (t1 - t0) / (iters - 1)
            best = per if best is None else min(best, per)
        return best * 1e9

    _cache["runner"] = run
    _cache["bench"] = bench
    run.bench = bench
    return run


def _make_in_maps(x, W1, b1, W2, b2, bias):
    wts = _build_weights(np.asarray(W1, np.float32), np.asarray(b1, np.float32),
                         np.asarray(W2, np.float32), np.asarray(b2, np.float32),
                         np.asarray(bias, np.float32))
    x = np.asarray(x, np.float32)
    xp = np.pad(x, ((0, 0), (0, 0), (1, 1), (1, 3)))  # [4, 16, 258, 260]
    in_maps = []
    for core in range(NCORES):
        b, h = divmod(core, 2)
        slab = np.ascontiguousarray(xp[b, :, 128 * h:128 * h + SLAB_ROWS, :])
        in_maps.append({"xs": slab, **wts})
    return in_maps


def kernel(x, W1, b1, W2, b2, bias):
    run = _get_runner()
    in_maps = _make_in_maps(x, W1, b1, W2, b2, bias)
    results = run(in_maps)
    out = np.empty((B, O, H, W), np.float32)
    for core in range(NCORES):
        b, h = divmod(core, 2)
        out[b, :, 128 * h:128 * h + 128, :] = results[core]["ys"]
    return out
